# revision 2
# baseline (speedup 1.0000x reference)
"""CFN cell (theta/eta gated RNN cell) on 8 Trainium2 NeuronCores.

Math (per reference):
    theta = sigmoid(state @ theta_u_w + inputs @ theta_w_w + theta_w_b)
    eta   = sigmoid(state @ eta_u_w   + inputs @ eta_w_w   + eta_w_b)
    Wx    = inputs @ wx_w
    h     = theta * tanh(state) + eta * tanh(Wx)
    return (h, h)

Sharding: tensor-parallel over the hidden dim H across the 8 cores (each
core owns a 256-column slice of all five weight matrices and computes
h[:, c*256:(c+1)*256] for the full batch).  All compute is done in the
transposed layout (h^T[h, b]) so that the contraction dim (d_in / d_state)
lands on the SBUF partition axis for both matmul operands:

    preact^T[h, b] = sum_k W[k, h] * actT[k, b]
    -> matmul(psum[h,128 x b,BW], lhsT=W[k128, h128], rhs=actT[k128, bBW])

The host passes X^T and S^T so no on-chip transposes are needed.  Matmul
operands use float32r (full fp32 data; PE streams 1 row/cycle for moving
free dim >= 256, vs 4 cycles/row for plain fp32).
"""

import numpy as np
from contextlib import ExitStack

import concourse.bass as bass
import concourse.mybir as mybir
import concourse.tile as tile
from concourse import bacc
from concourse.bass_utils import run_bass_kernel_spmd

F32 = mybir.dt.float32
F32R = mybir.dt.float32r
AF = mybir.ActivationFunctionType

B, D_IN, H, NCORES = 4096, 2048, 2048, 8
H_LOC = H // NCORES  # 256
BW = 256             # batch window = matmul moving free dim

TRACE = False
LAST_RESULTS = None
_NC_CACHE = {}


def build(nc, b, d_in, d_state, h_loc, bw):
    """Emit the per-core program. All shapes must be multiples of 128 (b of bw)."""
    n_ht, n_bw = h_loc // 128, b // bw
    ktx, kts = d_in // 128, d_state // 128

    xt = nc.dram_tensor("xt", [d_in, b], F32R, kind="ExternalInput").ap()
    st = nc.dram_tensor("st", [d_state, b], F32R, kind="ExternalInput").ap()
    sth = nc.dram_tensor("sth", [h_loc, b], F32, kind="ExternalInput").ap()
    wtu = nc.dram_tensor("wtu", [d_state, h_loc], F32R, kind="ExternalInput").ap()
    wtw = nc.dram_tensor("wtw", [d_in, h_loc], F32R, kind="ExternalInput").ap()
    weu = nc.dram_tensor("weu", [d_state, h_loc], F32R, kind="ExternalInput").ap()
    wew = nc.dram_tensor("wew", [d_in, h_loc], F32R, kind="ExternalInput").ap()
    wwx = nc.dram_tensor("wwx", [d_in, h_loc], F32R, kind="ExternalInput").ap()
    bt = nc.dram_tensor("bt", [h_loc], F32, kind="ExternalInput").ap()
    be = nc.dram_tensor("be", [h_loc], F32, kind="ExternalInput").ap()
    out = nc.dram_tensor("ht_out", [h_loc, b], F32, kind="ExternalOutput").ap()

    with tile.TileContext(nc) as tc, ExitStack() as ctx:
        consts = ctx.enter_context(tc.tile_pool(name="consts", bufs=1))
        acts = ctx.enter_context(tc.tile_pool(name="acts", bufs=2))
        temps = ctx.enter_context(tc.tile_pool(name="temps", bufs=3))
        outs = ctx.enter_context(tc.tile_pool(name="outs", bufs=3))
        psum = ctx.enter_context(tc.tile_pool(name="psum", bufs=2, space="PSUM"))

        # Resident weight slices: [128, kt, h_loc], k-tile-major along free dim.
        w_sb = {}
        for name, ap_, kt in (
            ("wtu", wtu, kts),
            ("wtw", wtw, ktx),
            ("weu", weu, kts),
            ("wew", wew, ktx),
            ("wwx", wwx, ktx),
        ):
            t = consts.tile([128, kt, h_loc], F32R, tag=name)
            nc.sync.dma_start(out=t, in_=ap_.rearrange("(t p) h -> p t h", p=128))
            w_sb[name] = t
        bt_sb = consts.tile([128, n_ht], F32, tag="bt")
        nc.sync.dma_start(out=bt_sb, in_=bt.rearrange("(t p) -> p t", p=128))
        be_sb = consts.tile([128, n_ht], F32, tag="be")
        nc.sync.dma_start(out=be_sb, in_=be.rearrange("(t p) -> p t", p=128))

        xt_r = xt.rearrange("(t p) n -> p t n", p=128)
        st_r = st.rearrange("(t p) n -> p t n", p=128)
        sth_r = sth.rearrange("(t p) n -> p t n", p=128)

        for ib in range(n_bw):
            bsl = slice(ib * bw, (ib + 1) * bw)
            xw = acts.tile([128, ktx, bw], F32R, tag="xw")
            nc.sync.dma_start(out=xw, in_=xt_r[:, :, bsl])
            sw = acts.tile([128, kts, bw], F32R, tag="sw")
            nc.sync.dma_start(out=sw, in_=st_r[:, :, bsl])
            shw = acts.tile([128, n_ht, bw], F32, tag="shw")
            nc.sync.dma_start(out=shw, in_=sth_r[:, :, bsl])

            for ih in range(n_ht):
                hsl = slice(ih * 128, (ih + 1) * 128)

                ps_t = psum.tile([128, bw], F32, tag="ps_t")
                for k in range(kts):
                    nc.tensor.matmul(
                        ps_t, w_sb["wtu"][:, k, hsl], sw[:, k, :],
                        start=(k == 0), stop=False,
                    )
                for k in range(ktx):
                    nc.tensor.matmul(
                        ps_t, w_sb["wtw"][:, k, hsl], xw[:, k, :],
                        start=False, stop=(k == ktx - 1),
                    )

                ps_e = psum.tile([128, bw], F32, tag="ps_e")
                for k in range(kts):
                    nc.tensor.matmul(
                        ps_e, w_sb["weu"][:, k, hsl], sw[:, k, :],
                        start=(k == 0), stop=False,
                    )
                for k in range(ktx):
                    nc.tensor.matmul(
                        ps_e, w_sb["wew"][:, k, hsl], xw[:, k, :],
                        start=False, stop=(k == ktx - 1),
                    )

                ps_w = psum.tile([128, bw], F32, tag="ps_w")
                for k in range(ktx):
                    nc.tensor.matmul(
                        ps_w, w_sb["wwx"][:, k, hsl], xw[:, k, :],
                        start=(k == 0), stop=(k == ktx - 1),
                    )

                theta = temps.tile([128, bw], F32, tag="theta")
                nc.scalar.activation(theta, ps_t, AF.Sigmoid, bias=bt_sb[:, ih : ih + 1])
                eta = temps.tile([128, bw], F32, tag="eta")
                nc.scalar.activation(eta, ps_e, AF.Sigmoid, bias=be_sb[:, ih : ih + 1])
                twx = temps.tile([128, bw], F32, tag="twx")
                nc.scalar.activation(twx, ps_w, AF.Tanh)
                tsh = temps.tile([128, bw], F32, tag="tsh")
                nc.scalar.activation(tsh, shw[:, ih, :], AF.Tanh)

                p1 = temps.tile([128, bw], F32, tag="p1")
                nc.vector.tensor_mul(p1, theta, tsh)
                p2 = temps.tile([128, bw], F32, tag="p2")
                nc.vector.tensor_mul(p2, eta, twx)
                ho = outs.tile([128, bw], F32, tag="ho")
                nc.vector.tensor_add(ho, p1, p2)
                nc.sync.dma_start(out=out[hsl, bsl], in_=ho)

    nc.compile()
    return nc


def _get_nc():
    key = (B, D_IN, H, H_LOC, BW)
    if key not in _NC_CACHE:
        nc = bacc.Bacc("TRN2", target_bir_lowering=False, debug=False,
                       num_devices=NCORES)
        _NC_CACHE[key] = build(nc, B, D_IN, H, H_LOC, BW)
    return _NC_CACHE[key]


def make_in_maps(inputs):
    x = np.ascontiguousarray(np.asarray(inputs["inputs"], dtype=np.float32))
    s = np.ascontiguousarray(np.asarray(inputs["state"], dtype=np.float32))
    w = {
        k: np.asarray(inputs[k], dtype=np.float32)
        for k in ("theta_u_w", "theta_w_w", "eta_u_w", "eta_w_w", "wx_w")
    }
    bt_full = np.asarray(inputs["theta_w_b"], dtype=np.float32)
    be_full = np.asarray(inputs["eta_w_b"], dtype=np.float32)

    xt = np.ascontiguousarray(x.T)  # [D_IN, B]
    st = np.ascontiguousarray(s.T)  # [H, B]

    in_maps = []
    for c in range(NCORES):
        hsl = slice(c * H_LOC, (c + 1) * H_LOC)
        in_maps.append({
            "xt": xt,
            "st": st,
            "sth": np.ascontiguousarray(st[hsl]),
            "wtu": np.ascontiguousarray(w["theta_u_w"][:, hsl]),
            "wtw": np.ascontiguousarray(w["theta_w_w"][:, hsl]),
            "weu": np.ascontiguousarray(w["eta_u_w"][:, hsl]),
            "wew": np.ascontiguousarray(w["eta_w_w"][:, hsl]),
            "wwx": np.ascontiguousarray(w["wx_w"][:, hsl]),
            "bt": np.ascontiguousarray(bt_full[hsl]),
            "be": np.ascontiguousarray(be_full[hsl]),
        })
    return in_maps


def kernel(**inputs):
    global LAST_RESULTS
    in_maps = make_in_maps(inputs)
    nc = _get_nc()
    res = run_bass_kernel_spmd(nc, in_maps, core_ids=list(range(NCORES)),
                               trace=TRACE)
    LAST_RESULTS = res

    hT = np.concatenate([res.results[c]["ht_out"] for c in range(NCORES)], axis=0)
    h = np.ascontiguousarray(hT.T)  # [B, H]
    return (h, h)
